# revision 2
# baseline (speedup 1.0000x reference)
"""Trainium2 Bass kernel for nn_Attention_3478923510454 (sparse_attention).

8-head attention over knowledge + shared transform MLP, sharded head-per-core
across 8 NeuronCores.

Per core h:
  S_h = sentences @ Ws[h] + bs[h]          (computed transposed: S_T [128e, Ns])
  K_h = knowledge @ Wk[h] + bk[h]          (K_T [128e, Nk])
  scores = S_h @ K_h^T / sqrt(128)         (fp32 PE matmul, [n,m] tiles)
  p = exp(scores)  (no max-subtraction: |scores| < 3)   + fused row sums (ACT)
  attn = p * mask[m] * (1/sum)             (one DVE scalar_tensor_tensor, in place)
  weights_out[nt] = attn                   (fp32, per-head graded output)
  attnT blocks via PE transpose, cast bf16
  head_out = attn @ knowledge              (bf16 x bf16, fp32 PSUM accum)
  partial = head_out @ W1[h]               (bf16, via PE-transposed head_out)
  ReduceScatter(partial) over 8 cores      (4 row-group chunks, overlapped)
  out rows = relu(p_shard + b1) @ W2 + b2  (fp32 tail on the core's row shard)

Host side shards the per-head parameters, runs SPMD on cores 0-7 and
reassembles (output [4096,1024], weights [32768,4096]).
"""

import math

import numpy as np

import concourse.bass as bass
import concourse.mybir as mybir
import concourse.tile as tile
from concourse import bacc
from concourse.bass_utils import run_bass_kernel_spmd
from concourse.masks import make_identity

F32 = mybir.dt.float32
BF16 = mybir.dt.bfloat16
P = 128

# Problem constants (hardcoded per the harness contract)
SENT_DIM = 1024
NUM_HEADS = 8
ALIGN = 128
N_SENT = 4096
N_KNOW = 4096
N_CORES = 8
RS_GROUPS = 4


def build_kernel(ns=N_SENT, nk=N_KNOW, world=N_CORES, rs_groups=RS_GROUPS):
    """Emit the per-core Bass program. Identical program on every core (SPMD);
    all per-head differences come in through the input tensors."""
    d = SENT_DIM
    dc = d // P                      # 8 contraction chunks of 128
    n_tiles = ns // P                # sentence row tiles
    m_tiles = nk // P                # knowledge row tiles (attn columns)
    shard_rows = ns // world         # output rows owned by this core
    group_rows = ns // rs_groups     # rows covered by one ReduceScatter chunk
    tiles_per_group = group_rows // P
    scale = 1.0 / math.sqrt(float(ALIGN))

    nc = bacc.Bacc(None, target_bir_lowering=False, debug=False)

    sent = nc.dram_tensor("sentences", [ns, d], F32, kind="ExternalInput")
    know = nc.dram_tensor("knowledge", [nk, d], F32, kind="ExternalInput")
    mask = nc.dram_tensor("mask", [1, nk], F32, kind="ExternalInput")
    Ws = nc.dram_tensor("Ws", [d, ALIGN], F32, kind="ExternalInput")
    bs = nc.dram_tensor("bs", [ALIGN, 1], F32, kind="ExternalInput")
    Wk = nc.dram_tensor("Wk", [d, ALIGN], F32, kind="ExternalInput")
    bk = nc.dram_tensor("bk", [ALIGN, 1], F32, kind="ExternalInput")
    W1 = nc.dram_tensor("W1", [d, d], F32, kind="ExternalInput")
    b1 = nc.dram_tensor("b1", [1, d], F32, kind="ExternalInput")
    W2 = nc.dram_tensor("W2", [d, d], F32, kind="ExternalInput")
    b2 = nc.dram_tensor("b2", [1, d], F32, kind="ExternalInput")

    weights_out = nc.dram_tensor("weights_out", [ns, nk], F32, kind="ExternalOutput")
    out_shard = nc.dram_tensor("out_shard", [shard_rows, d], F32, kind="ExternalOutput")

    Exp = mybir.ActivationFunctionType.Exp
    Relu = mybir.ActivationFunctionType.Relu
    mult = mybir.AluOpType.mult
    add = mybir.AluOpType.add
    AX = mybir.AxisListType.X

    with tile.TileContext(nc) as tc:
        with (
            tc.tile_pool(name="dram", bufs=1, space="DRAM") as dram,
            tc.tile_pool(name="psc", bufs=2, space="PSUM") as psc,
            tc.tile_pool(name="ptp", bufs=1, space="PSUM") as ptp,
            tc.tile_pool(name="php", bufs=1, space="PSUM") as php,
            tc.tile_pool(name="small", bufs=4) as small,
        ):
            partial_d = dram.tile([ns, d], F32)
            rs_d = dram.tile([shard_rows, d], F32)

            with (
                tc.tile_pool(name="const", bufs=1) as const,
                tc.tile_pool(name="big", bufs=1) as big,
                tc.tile_pool(name="t1024", bufs=4) as t1024,
                tc.tile_pool(name="work", bufs=1) as work,
            ):
                ident = const.tile([P, P], F32)
                make_identity(nc, ident)
                mask_bc = const.tile([P, nk], F32)
                nc.gpsimd.dma_start(mask_bc[:], mask[:].partition_broadcast(P))
                Ws_sb = const.tile([P, dc * ALIGN], F32)
                Wk_sb = const.tile([P, dc * ALIGN], F32)
                bs_sb = const.tile([P, 1], F32)
                bk_sb = const.tile([P, 1], F32)
                nc.sync.dma_start(bs_sb[:], bs[:])
                nc.sync.dma_start(bk_sb[:], bk[:])
                for c in range(dc):
                    nc.sync.dma_start(
                        Ws_sb[:, c * ALIGN : (c + 1) * ALIGN],
                        Ws[c * P : (c + 1) * P, :],
                    )
                    nc.sync.dma_start(
                        Wk_sb[:, c * ALIGN : (c + 1) * ALIGN],
                        Wk[c * P : (c + 1) * P, :],
                    )

                S_T = big.tile([P, ns], F32)
                K_T = big.tile([P, nk], F32)
                know_bf = big.tile([P, m_tiles * d], BF16)
                W1_bf = big.tile([P, dc * d], BF16)
                for c in range(dc):
                    wtmp = t1024.tile([P, d], F32, tag="t1024")
                    nc.sync.dma_start(wtmp[:], W1[c * P : (c + 1) * P, :])
                    nc.vector.tensor_copy(W1_bf[:, c * d : (c + 1) * d], wtmp[:])

                # ---- projections: build S_T / K_T via per-tile PE transpose ----
                def project(src, n_src_tiles, Wp_sb, bias_sb, dstT, cast_dst):
                    for t in range(n_src_tiles):
                        x = t1024.tile([P, d], F32, tag="t1024")
                        nc.sync.dma_start(x[:], src[t * P : (t + 1) * P, :])
                        if cast_dst is not None:
                            nc.vector.tensor_copy(
                                cast_dst[:, t * d : (t + 1) * d], x[:]
                            )
                        tp = (ptp if t % 2 == 0 else php).tile(
                            [P, d], F32, tag="tp" if t % 2 == 0 else "hp"
                        )
                        for c in range(dc):
                            nc.tensor.transpose(
                                tp[:, c * P : (c + 1) * P],
                                x[:, c * P : (c + 1) * P],
                                ident[:],
                            )
                        xT = t1024.tile([P, d], F32, tag="t1024")
                        nc.scalar.copy(xT[:], tp[:])
                        pacc = psc.tile([P, P], F32, tag="scores")
                        for c in range(dc):
                            nc.tensor.matmul(
                                pacc[:],
                                Wp_sb[:, c * ALIGN : (c + 1) * ALIGN],
                                xT[:, c * P : (c + 1) * P],
                                start=(c == 0),
                                stop=(c == dc - 1),
                            )
                        nc.vector.tensor_scalar_add(
                            dstT[:, t * P : (t + 1) * P], pacc[:], bias_sb[:]
                        )

                project(sent, n_tiles, Ws_sb, bs_sb, S_T, None)
                project(know, m_tiles, Wk_sb, bk_sb, K_T, know_bf)

                # ---- main fused loop over sentence row tiles ----
                n_q = max(1, nk // 1024)      # score PSUM chunks of <=1024 cols
                qw = nk // n_q                # columns per chunk (<= 1024)
                attnT = work.tile([P, m_tiles * P], BF16)
                hoT = work.tile([P, d], BF16)

                for nt in range(n_tiles):
                    ssl = S_T[:, nt * P : (nt + 1) * P]
                    p_t = work.tile([P, nk], F32, tag="p")
                    sums_q = small.tile([P, n_q], F32, tag="sums_q")
                    for q in range(n_q):
                        sc = psc.tile([P, qw], F32, tag="scores")
                        for j in range(qw // 512):
                            nc.tensor.matmul(
                                sc[:, j * 512 : (j + 1) * 512],
                                ssl,
                                K_T[:, q * qw + j * 512 : q * qw + (j + 1) * 512],
                                start=True,
                                stop=True,
                            )
                        nc.scalar.activation(
                            p_t[:, q * qw : (q + 1) * qw],
                            sc[:],
                            Exp,
                            scale=scale,
                            accum_out=sums_q[:, q : q + 1],
                        )
                    sums = small.tile([P, 1], F32, tag="sums")
                    nc.vector.reduce_sum(sums[:], sums_q[:], axis=AX)
                    recip = small.tile([P, 1], F32, tag="recip")
                    nc.vector.reciprocal(recip[:], sums[:])
                    # attn = p * (1/sum) * mask, in place
                    nc.vector.scalar_tensor_tensor(
                        p_t[:], p_t[:], recip[:], mask_bc[:], op0=mult, op1=mult
                    )
                    nc.sync.dma_start(weights_out[nt * P : (nt + 1) * P, :], p_t[:])

                    # transpose attn into bf16 [m, n] blocks (stages of 8)
                    for s in range(m_tiles // 8):
                        tp = (ptp if s % 2 == 0 else php).tile(
                            [P, 8 * P], F32, tag="tp" if s % 2 == 0 else "hp"
                        )
                        for k in range(8):
                            mc = s * 8 + k
                            nc.tensor.transpose(
                                tp[:, k * P : (k + 1) * P],
                                p_t[:, mc * P : (mc + 1) * P],
                                ident[:],
                            )
                        eng = nc.scalar if s % 2 == 0 else nc.vector
                        if s % 2 == 0:
                            nc.scalar.copy(
                                attnT[:, s * 8 * P : (s + 1) * 8 * P], tp[:]
                            )
                        else:
                            nc.vector.tensor_copy(
                                attnT[:, s * 8 * P : (s + 1) * 8 * P], tp[:]
                            )

                    # head_out[nt] = attn @ knowledge   (bf16)
                    ho = php.tile([P, d], F32, tag="hp")
                    for mc in range(m_tiles):
                        for j in range(d // 512):
                            nc.tensor.matmul(
                                ho[:, j * 512 : (j + 1) * 512],
                                attnT[:, mc * P : (mc + 1) * P],
                                know_bf[:, mc * d + j * 512 : mc * d + (j + 1) * 512],
                                start=(mc == 0),
                                stop=(mc == m_tiles - 1),
                            )
                    ho_sb = t1024.tile([P, d], F32, tag="t1024")
                    nc.scalar.copy(ho_sb[:], ho[:])

                    # partial[nt] = head_out @ W1_h   (bf16, via transposed ho)
                    tp = ptp.tile([P, d], F32, tag="tp")
                    for c in range(dc):
                        nc.tensor.transpose(
                            tp[:, c * P : (c + 1) * P],
                            ho_sb[:, c * P : (c + 1) * P],
                            ident[:],
                        )
                    nc.scalar.copy(hoT[:], tp[:])
                    part = php.tile([P, d], F32, tag="hp")
                    for c in range(dc):
                        for j in range(d // 512):
                            nc.tensor.matmul(
                                part[:, j * 512 : (j + 1) * 512],
                                hoT[:, c * P : (c + 1) * P],
                                W1_bf[:, c * d + j * 512 : c * d + (j + 1) * 512],
                                start=(c == 0),
                                stop=(c == dc - 1),
                            )
                    part_sb = t1024.tile([P, d], F32, tag="t1024")
                    nc.vector.tensor_copy(part_sb[:], part[:])
                    nc.sync.dma_start(
                        partial_d[nt * P : (nt + 1) * P, :], part_sb[:]
                    )

                    # fire the ReduceScatter chunk once its row group is done
                    if (nt + 1) % tiles_per_group == 0:
                        g = (nt + 1) // tiles_per_group - 1
                        gr = group_rows
                        grw = gr // world
                        nc.gpsimd.collective_compute(
                            "ReduceScatter",
                            add,
                            replica_groups=[list(range(world))],
                            ins=[partial_d[g * gr : (g + 1) * gr, :]],
                            outs=[rs_d[g * grw : (g + 1) * grw, :]],
                        )

            # ---- tail: out rows = relu(p_shard + b1) @ W2 + b2 ----
            with (
                tc.tile_pool(name="tailc", bufs=1) as tailc,
                tc.tile_pool(name="tailw", bufs=4) as tailw,
            ):
                ident2 = tailc.tile([P, P], F32)
                make_identity(nc, ident2)
                b1_bc = tailc.tile([P, d], F32)
                b2_bc = tailc.tile([P, d], F32)
                nc.gpsimd.dma_start(b1_bc[:], b1[:].partition_broadcast(P))
                nc.gpsimd.dma_start(b2_bc[:], b2[:].partition_broadcast(P))
                W2_sb = tailc.tile([P, dc * d], F32)
                for c in range(dc):
                    nc.sync.dma_start(
                        W2_sb[:, c * d : (c + 1) * d], W2[c * P : (c + 1) * P, :]
                    )
                for t in range(shard_rows // P):
                    pg = tailw.tile([P, d], F32, tag="tw")
                    nc.sync.dma_start(pg[:], rs_d[t * P : (t + 1) * P, :])
                    hrelu = tailw.tile([P, d], F32, tag="tw")
                    # relu(p + b1): bias lives along the free dim -> TT add + relu
                    nc.vector.scalar_tensor_tensor(
                        hrelu[:], pg[:], 0.0, b1_bc[:], op0=add, op1=add
                    )
                    nc.scalar.activation(hrelu[:], hrelu[:], Relu)
                    tp = ptp.tile([P, d], F32, tag="tp")
                    for c in range(dc):
                        nc.tensor.transpose(
                            tp[:, c * P : (c + 1) * P],
                            hrelu[:, c * P : (c + 1) * P],
                            ident2[:],
                        )
                    hT = tailw.tile([P, d], F32, tag="tw")
                    nc.scalar.copy(hT[:], tp[:])
                    ops = php.tile([P, d], F32, tag="hp")
                    for c in range(dc):
                        for j in range(d // 512):
                            nc.tensor.matmul(
                                ops[:, j * 512 : (j + 1) * 512],
                                hT[:, c * P : (c + 1) * P],
                                W2_sb[:, c * d + j * 512 : c * d + (j + 1) * 512],
                                start=(c == 0),
                                stop=(c == dc - 1),
                            )
                    ob = tailw.tile([P, d], F32, tag="tw")
                    nc.vector.scalar_tensor_tensor(
                        ob[:], ops[:], 0.0, b2_bc[:], op0=add, op1=add
                    )
                    nc.sync.dma_start(out_shard[t * P : (t + 1) * P, :], ob[:])

    nc.compile()
    return nc


_NC_CACHE = {}


def _get_nc(key=(N_SENT, N_KNOW, N_CORES, RS_GROUPS)):
    if key not in _NC_CACHE:
        _NC_CACHE[key] = build_kernel(*key)
    return _NC_CACHE[key]


def make_in_maps(inputs, world=N_CORES):
    sent = np.ascontiguousarray(inputs["sentences"], dtype=np.float32)
    know = np.ascontiguousarray(inputs["knowledge"], dtype=np.float32)
    mask = np.ascontiguousarray(inputs["mask"], dtype=np.float32).reshape(1, -1)
    W1 = np.ascontiguousarray(inputs["W1"], dtype=np.float32)
    b1 = np.ascontiguousarray(inputs["b1"], dtype=np.float32).reshape(1, -1)
    W2 = np.ascontiguousarray(inputs["W2"], dtype=np.float32)
    b2 = np.ascontiguousarray(inputs["b2"], dtype=np.float32).reshape(1, -1)
    d = W2.shape[0]
    in_maps = []
    for h in range(world):
        in_maps.append(
            {
                "sentences": sent,
                "knowledge": know,
                "mask": mask,
                "Ws": np.ascontiguousarray(inputs["Ws"][h], dtype=np.float32),
                "bs": np.ascontiguousarray(inputs["bs"][h], dtype=np.float32).reshape(-1, 1),
                "Wk": np.ascontiguousarray(inputs["Wk"][h], dtype=np.float32),
                "bk": np.ascontiguousarray(inputs["bk"][h], dtype=np.float32).reshape(-1, 1),
                "W1": np.ascontiguousarray(W1[h * d : (h + 1) * d], dtype=np.float32),
                "b1": b1,
                "W2": W2,
                "b2": b2,
            }
        )
    return in_maps


def assemble(results, ns=N_SENT, world=N_CORES, rs_groups=RS_GROUPS, d=SENT_DIM):
    weights = np.concatenate([results[h]["weights_out"] for h in range(world)], axis=0)
    output = np.empty((ns, d), dtype=np.float32)
    grw = ns // rs_groups // world  # rows per (group, core)
    for i in range(world):
        sh = results[i]["out_shard"]
        for g in range(rs_groups):
            output[g * (ns // rs_groups) + i * grw : g * (ns // rs_groups) + (i + 1) * grw] = sh[
                g * grw : (g + 1) * grw
            ]
    return output, weights


def kernel(**inputs):
    nc = _get_nc()
    in_maps = make_in_maps(inputs)
    res = run_bass_kernel_spmd(nc, in_maps, core_ids=list(range(N_CORES)))
    return assemble(res.results)


# revision 15
# speedup vs baseline: 1.5052x; 1.5052x over previous
"""Trainium2 Bass kernel for nn_Attention_3478923510454 (sparse_attention).

8-head attention over knowledge + shared transform MLP, sharded head-per-core
across 8 NeuronCores.

Per core h:
  S_h = sentences @ Ws[h] + bs[h]          (fp16 matmuls; S_T [128e, Ns] fp16)
  K_h = knowledge @ Wk[h] + bk[h]          (K_T [128e, Nk] fp16)
  scores = S_h @ K_h^T / sqrt(128)         (fp16 PE matmul -> fp32 PSUM)
  p = exp(scores)                          (ACT, fused row-sum accum_out;
                                            |scores| < 3 so no max-subtraction)
  weights_out = p * (1/sum) * mask         (fp32, one DVE op in place)
  q = p * mask (fp16)                      (transposed by PE in fp16)
  head_out = (q^T)^T @ knowledge * (1/sum) (fp16 matmul, scale on PSUM evict)
  partial = head_out @ W1[h]               (fp16, via PE-transposed head_out)
  ReduceScatter(partial) over 8 cores      (row-group chunks, overlapped)
  out rows = relu(p_shard + b1) @ W2 + b2  (fp16 matmul tail on row shard)

Host side shards the per-head parameters, runs SPMD on cores 0-7 and
reassembles (output [4096,1024], weights [32768,4096]).
"""

import math

import numpy as np

import concourse.bass as bass
import concourse.mybir as mybir
import concourse.tile as tile
from concourse import bacc
from concourse.bass_utils import run_bass_kernel_spmd
from concourse.masks import make_identity

F32 = mybir.dt.float32
F16 = mybir.dt.float16
P = 128

# Problem constants (hardcoded per the harness contract)
SENT_DIM = 1024
NUM_HEADS = 8
ALIGN = 128
N_SENT = 4096
N_KNOW = 4096
N_CORES = 8
RS_GROUPS = 8


def build_kernel(ns=N_SENT, nk=N_KNOW, world=N_CORES, rs_groups=RS_GROUPS, reps=1):
    """Emit the per-core Bass program. Identical program on every core (SPMD);
    all per-head differences come in through the input tensors.

    reps > 1 emits the body multiple times (timing: body = diff / (reps-1))."""
    d = SENT_DIM
    dc = d // P                      # 8 contraction chunks of 128
    n_tiles = ns // P                # sentence row tiles
    m_tiles = nk // P                # knowledge row tiles (attn columns)
    shard_rows = ns // world         # output rows owned by this core
    group_rows = ns // rs_groups     # rows covered by one ReduceScatter chunk
    tiles_per_group = group_rows // P
    scale = 1.0 / math.sqrt(float(ALIGN))

    nc = bacc.Bacc(None, target_bir_lowering=False, debug=False)

    sent = nc.dram_tensor("sentences", [ns, d], F32, kind="ExternalInput")
    know = nc.dram_tensor("knowledge", [nk, d], F32, kind="ExternalInput")
    mask = nc.dram_tensor("mask", [1, nk], F32, kind="ExternalInput")
    Ws = nc.dram_tensor("Ws", [d, ALIGN], F32, kind="ExternalInput")
    bs = nc.dram_tensor("bs", [ALIGN, 1], F32, kind="ExternalInput")
    Wk = nc.dram_tensor("Wk", [d, ALIGN], F32, kind="ExternalInput")
    bk = nc.dram_tensor("bk", [ALIGN, 1], F32, kind="ExternalInput")
    W1 = nc.dram_tensor("W1", [d, d], F32, kind="ExternalInput")
    b1 = nc.dram_tensor("b1", [1, d], F32, kind="ExternalInput")
    W2 = nc.dram_tensor("W2", [d, d], F32, kind="ExternalInput")
    b2 = nc.dram_tensor("b2", [1, d], F32, kind="ExternalInput")

    weights_out = nc.dram_tensor("weights_out", [ns, nk], F32, kind="ExternalOutput")
    out_shard = nc.dram_tensor("out_shard", [shard_rows, d], F32, kind="ExternalOutput")

    Exp = mybir.ActivationFunctionType.Exp
    Relu = mybir.ActivationFunctionType.Relu
    mult = mybir.AluOpType.mult
    add = mybir.AluOpType.add
    AX = mybir.AxisListType.X

    with tile.TileContext(nc) as tc:
        with (
            tc.tile_pool(name="dram", bufs=1, space="DRAM") as dram,
            tc.tile_pool(name="psc", bufs=2, space="PSUM") as psc,
            tc.tile_pool(name="ptp", bufs=2, space="PSUM") as ptp,
            tc.tile_pool(name="php", bufs=1, space="PSUM") as php,
            tc.tile_pool(name="small", bufs=4) as small,
        ):
            for _rep in range(reps):
                partial_d = dram.tile([ns, d], F32)
                rs_d = dram.tile([shard_rows, d], F32)

                with (
                    tc.tile_pool(name="const", bufs=1) as const,
                    tc.tile_pool(name="big", bufs=1) as big,
                    tc.tile_pool(name="t16", bufs=3) as t16,
                    tc.tile_pool(name="work", bufs=1) as work,
                ):
                    ident = const.tile([P, P], F16)
                    make_identity(nc, ident)
                    Ws_sb = const.tile([P, dc * ALIGN], F16)
                    Wk_sb = const.tile([P, dc * ALIGN], F16)
                    bs_sb = const.tile([P, 1], F32)
                    bk_sb = const.tile([P, 1], F32)
                    nc.sync.dma_start(bs_sb[:], bs[:])
                    nc.sync.dma_start(bk_sb[:], bk[:])
                    # whole-parameter casting loads (SWDGE casts fp32 -> fp16)
                    nc.gpsimd.dma_start(
                        Ws_sb[:], Ws[:].rearrange("(a p) e -> p a e", p=P)
                    )
                    nc.gpsimd.dma_start(
                        Wk_sb[:], Wk[:].rearrange("(a p) e -> p a e", p=P)
                    )

                    S_T = big.tile([P, ns], F16)
                    K_T = big.tile([P, nk], F16)
                    know_16 = big.tile([P, m_tiles * d], F16)
                    W1_16 = big.tile([P, dc * d], F16)
                    nc.gpsimd.dma_start(
                        W1_16[:], W1[:].rearrange("(a p) e -> p a e", p=P)
                    )

                    # ---- projections: S_T / K_T via fp16 PE transpose + matmul ----
                    def project(x16, Wp_sb, bias_sb, dstT, t):
                        # x16: [P, d] fp16 tile of source rows t*128..; writes
                        # dstT[:, t*128:(t+1)*128] = (x @ Wp + b)^T
                        tp = ptp.tile([P, d], F16, tag="tp")
                        for c in range(dc):
                            nc.tensor.transpose(
                                tp[:, c * P : (c + 1) * P],
                                x16[:, c * P : (c + 1) * P],
                                ident[:],
                            )
                        xT = t16.tile([P, d], F16, tag="t16")
                        nc.scalar.copy(xT[:], tp[:])
                        pacc = psc.tile([P, P], F32, tag="scores")
                        for c in range(dc):
                            nc.tensor.matmul(
                                pacc[:],
                                Wp_sb[:, c * ALIGN : (c + 1) * ALIGN],
                                xT[:, c * P : (c + 1) * P],
                                start=(c == 0),
                                stop=(c == dc - 1),
                            )
                        nc.vector.tensor_scalar_add(
                            dstT[:, t * P : (t + 1) * P], pacc[:], bias_sb[:]
                        )

                    BATCH = 4  # row tiles per casting DMA
                    for t0 in range(0, n_tiles, BATCH):
                        xs = t16.tile([P, BATCH * d], F16, tag="xs", bufs=2)
                        nc.gpsimd.dma_start(
                            xs[:],
                            sent[t0 * P : (t0 + BATCH) * P, :].rearrange(
                                "(a p) e -> p a e", p=P
                            ),
                        )
                        for i in range(BATCH):
                            project(
                                xs[:, i * d : (i + 1) * d], Ws_sb, bs_sb, S_T, t0 + i
                            )
                    for t0 in range(0, m_tiles, BATCH):
                        nc.gpsimd.dma_start(
                            know_16[:, t0 * d : (t0 + BATCH) * d],
                            know[t0 * P : (t0 + BATCH) * P, :].rearrange(
                                "(a p) e -> p a e", p=P
                            ),
                        )
                        for i in range(BATCH):
                            project(
                                know_16[:, (t0 + i) * d : (t0 + i + 1) * d],
                                Wk_sb,
                                bk_sb,
                                K_T,
                                t0 + i,
                            )

                    # mask broadcast (emitted late: gpsimd queue is busy with
                    # the casting loads first; mask is only needed at tile 0
                    # of the main loop)
                    mask_bc = const.tile([P, nk], F32)
                    nc.gpsimd.dma_start(mask_bc[:], mask[:].partition_broadcast(P))

                    # ---- main fused loop over sentence row tiles ----
                    n_q = max(1, nk // 1024)   # score PSUM chunks of <=1024 cols
                    qw = nk // n_q
                    attnT = work.tile([P, m_tiles * P], F16)
                    hoT = work.tile([P, d], F16)

                    for nt in range(n_tiles):
                        ssl = S_T[:, nt * P : (nt + 1) * P]
                        p_t = work.tile([P, nk], F32, tag="p", bufs=2)
                        q_t = work.tile([P, nk], F16, tag="q")
                        sums_q = small.tile([P, n_q], F32, tag="sums_q")
                        for q in range(n_q):
                            sc = psc.tile([P, qw], F32, tag="scores")
                            for j in range(qw // 512):
                                nc.tensor.matmul(
                                    sc[:, j * 512 : (j + 1) * 512],
                                    ssl,
                                    K_T[:, q * qw + j * 512 : q * qw + (j + 1) * 512],
                                    start=True,
                                    stop=True,
                                )
                            nc.scalar.activation(
                                p_t[:, q * qw : (q + 1) * qw],
                                sc[:],
                                Exp,
                                scale=scale,
                                accum_out=sums_q[:, q : q + 1],
                            )
                        sums = small.tile([P, 1], F32, tag="sums")
                        nc.vector.reduce_sum(sums[:], sums_q[:], axis=AX)
                        recip = small.tile([P, 1], F32, tag="recip")
                        nc.vector.reciprocal(recip[:], sums[:])
                        # q = p * mask (fp16) -- un-normalized masked weights
                        nc.vector.tensor_mul(q_t[:], p_t[:], mask_bc[:])
                        # weights output = p * (1/sum) * mask (fp32, in place)
                        nc.vector.scalar_tensor_tensor(
                            p_t[:], p_t[:], recip[:], mask_bc[:], op0=mult, op1=mult
                        )
                        nc.sync.dma_start(
                            weights_out[nt * P : (nt + 1) * P, :], p_t[:]
                        )

                        # transpose q into fp16 [m, n] blocks (stages of 8)
                        for s in range(m_tiles // 8):
                            tp = ptp.tile([P, 8 * P], F16, tag="tp")
                            for k in range(8):
                                mc = s * 8 + k
                                nc.tensor.transpose(
                                    tp[:, k * P : (k + 1) * P],
                                    q_t[:, mc * P : (mc + 1) * P],
                                    ident[:],
                                )
                            nc.scalar.copy(
                                attnT[:, s * 8 * P : (s + 1) * 8 * P], tp[:]
                            )

                        # head_out[nt] = q^T.T @ knowledge, scaled by 1/sum
                        ho = php.tile([P, d], F32, tag="hp")
                        for mc in range(m_tiles):
                            for j in range(d // 512):
                                nc.tensor.matmul(
                                    ho[:, j * 512 : (j + 1) * 512],
                                    attnT[:, mc * P : (mc + 1) * P],
                                    know_16[
                                        :, mc * d + j * 512 : mc * d + (j + 1) * 512
                                    ],
                                    start=(mc == 0),
                                    stop=(mc == m_tiles - 1),
                                )
                        ho_sb = t16.tile([P, d], F16, tag="t16")
                        nc.vector.tensor_scalar_mul(ho_sb[:], ho[:], recip[:])

                        # partial[nt] = head_out @ W1_h (fp16)
                        tp = ptp.tile([P, d], F16, tag="tp")
                        for c in range(dc):
                            nc.tensor.transpose(
                                tp[:, c * P : (c + 1) * P],
                                ho_sb[:, c * P : (c + 1) * P],
                                ident[:],
                            )
                        nc.scalar.copy(hoT[:], tp[:])
                        part = php.tile([P, d], F32, tag="hp")
                        for c in range(dc):
                            for j in range(d // 512):
                                nc.tensor.matmul(
                                    part[:, j * 512 : (j + 1) * 512],
                                    hoT[:, c * P : (c + 1) * P],
                                    W1_16[:, c * d + j * 512 : c * d + (j + 1) * 512],
                                    start=(c == 0),
                                    stop=(c == dc - 1),
                                )
                        part_sb = small.tile([P, d], F32, tag="part", bufs=2)
                        nc.scalar.copy(part_sb[:], part[:])
                        nc.sync.dma_start(
                            partial_d[nt * P : (nt + 1) * P, :], part_sb[:]
                        )

                        # fire the ReduceScatter chunk once its row group is done
                        if (nt + 1) % tiles_per_group == 0:
                            g = (nt + 1) // tiles_per_group - 1
                            gr = group_rows
                            grw = gr // world
                            nc.gpsimd.collective_compute(
                                "ReduceScatter",
                                add,
                                replica_groups=[list(range(world))],
                                ins=[partial_d[g * gr : (g + 1) * gr, :]],
                                outs=[rs_d[g * grw : (g + 1) * grw, :]],
                            )

                # ---- tail: out rows = relu(p_shard + b1) @ W2 + b2 ----
                with (
                    tc.tile_pool(name="tailc", bufs=1) as tailc,
                    tc.tile_pool(name="tailw", bufs=4) as tailw,
                ):
                    ident2 = tailc.tile([P, P], F16)
                    make_identity(nc, ident2)
                    b1_bc = tailc.tile([P, d], F32)
                    b2_bc = tailc.tile([P, d], F32)
                    nc.gpsimd.dma_start(b1_bc[:], b1[:].partition_broadcast(P))
                    nc.gpsimd.dma_start(b2_bc[:], b2[:].partition_broadcast(P))
                    W2_16 = tailc.tile([P, dc * d], F16)
                    for c in range(dc):
                        nc.gpsimd.dma_start(
                            W2_16[:, c * d : (c + 1) * d], W2[c * P : (c + 1) * P, :]
                        )
                    for t in range(shard_rows // P):
                        pg = tailw.tile([P, d], F32, tag="tw")
                        nc.sync.dma_start(pg[:], rs_d[t * P : (t + 1) * P, :])
                        hsum = tailw.tile([P, d], F32, tag="tw")
                        nc.vector.scalar_tensor_tensor(
                            hsum[:], pg[:], 0.0, b1_bc[:], op0=add, op1=add
                        )
                        hrelu = tailw.tile([P, d], F16, tag="tw16")
                        nc.scalar.activation(hrelu[:], hsum[:], Relu)
                        tp = ptp.tile([P, d], F16, tag="tp")
                        for c in range(dc):
                            nc.tensor.transpose(
                                tp[:, c * P : (c + 1) * P],
                                hrelu[:, c * P : (c + 1) * P],
                                ident2[:],
                            )
                        hT = tailw.tile([P, d], F16, tag="tw16")
                        nc.scalar.copy(hT[:], tp[:])
                        ops = psc.tile([P, d], F32, tag="scores")
                        for c in range(dc):
                            for j in range(d // 512):
                                nc.tensor.matmul(
                                    ops[:, j * 512 : (j + 1) * 512],
                                    hT[:, c * P : (c + 1) * P],
                                    W2_16[:, c * d + j * 512 : c * d + (j + 1) * 512],
                                    start=(c == 0),
                                    stop=(c == dc - 1),
                                )
                        ob = tailw.tile([P, d], F32, tag="tw")
                        nc.vector.scalar_tensor_tensor(
                            ob[:], ops[:], 0.0, b2_bc[:], op0=add, op1=add
                        )
                        nc.sync.dma_start(out_shard[t * P : (t + 1) * P, :], ob[:])

    nc.compile()
    return nc


_NC_CACHE = {}


def _get_nc(key=(N_SENT, N_KNOW, N_CORES, RS_GROUPS)):
    if key not in _NC_CACHE:
        _NC_CACHE[key] = build_kernel(*key)
    return _NC_CACHE[key]


def make_in_maps(inputs, world=N_CORES):
    sent = np.ascontiguousarray(inputs["sentences"], dtype=np.float32)
    know = np.ascontiguousarray(inputs["knowledge"], dtype=np.float32)
    mask = np.ascontiguousarray(inputs["mask"], dtype=np.float32).reshape(1, -1)
    W1 = np.ascontiguousarray(inputs["W1"], dtype=np.float32)
    b1 = np.ascontiguousarray(inputs["b1"], dtype=np.float32).reshape(1, -1)
    W2 = np.ascontiguousarray(inputs["W2"], dtype=np.float32)
    b2 = np.ascontiguousarray(inputs["b2"], dtype=np.float32).reshape(1, -1)
    d = W2.shape[0]
    in_maps = []
    for h in range(world):
        in_maps.append(
            {
                "sentences": sent,
                "knowledge": know,
                "mask": mask,
                "Ws": np.ascontiguousarray(inputs["Ws"][h], dtype=np.float32),
                "bs": np.ascontiguousarray(inputs["bs"][h], dtype=np.float32).reshape(-1, 1),
                "Wk": np.ascontiguousarray(inputs["Wk"][h], dtype=np.float32),
                "bk": np.ascontiguousarray(inputs["bk"][h], dtype=np.float32).reshape(-1, 1),
                "W1": np.ascontiguousarray(W1[h * d : (h + 1) * d], dtype=np.float32),
                "b1": b1,
                "W2": W2,
                "b2": b2,
            }
        )
    return in_maps


def assemble(results, ns=N_SENT, world=N_CORES, rs_groups=RS_GROUPS, d=SENT_DIM):
    weights = np.concatenate([results[h]["weights_out"] for h in range(world)], axis=0)
    output = np.empty((ns, d), dtype=np.float32)
    grw = ns // rs_groups // world  # rows per (group, core)
    for i in range(world):
        sh = results[i]["out_shard"]
        for g in range(rs_groups):
            output[g * (ns // rs_groups) + i * grw : g * (ns // rs_groups) + (i + 1) * grw] = sh[
                g * grw : (g + 1) * grw
            ]
    return output, weights


def kernel(**inputs):
    nc = _get_nc()
    in_maps = make_in_maps(inputs)
    res = run_bass_kernel_spmd(nc, in_maps, core_ids=list(range(N_CORES)))
    return assemble(res.results)
